# revision 1
# baseline (speedup 1.0000x reference)
"""Trainium2 Bass kernel for nn_MultiHeadAttention_5059471475068.

Reference computation (B=2, N=2048, DIM=1024, H=16 heads, d=64):
    q = x @ Wq.T + bq ; k = x @ Wk.T + bk ; v = x @ Wv.T + bv   (per-head split)
    scores[h,b,n,m] = (k[h,b,n,:] . q[h,b,m,:]) / sqrt(DIM)
    attn = softmax(scores, axis=m)
    out[h,b,n,:] = attn @ v ; out = concat_heads @ Wo.T + bo

Sharding: 8 cores = 2 batches x 4 head-groups (4 heads per core).
Each core computes its heads' q,k,v projections, attention, and a partial
output projection (its heads' columns of the concat times the matching rows
of Wo.T).  Host sums the 4 partials per batch and adds bo (the unshard step
for the tensor-parallel dimension).

On-chip layout: scores are computed transposed (S^T[m, n], partition = m) so
that E = exp(S^T) feeds the attn@v matmul directly as the moving operand
with contraction over m, with no transposes.  The softmax denominator
(column sum of E) is folded into the attn@v matmul by appending a
ones-column to v (stationary operand [v | 1], M=65): PSUM row 64 of the
attn@v output accumulates sum_m E[m, n] exactly in fp32.

Schedule: one software pipeline keyed to the ScalarE exp stream (the
second-busiest engine).  q/k of head-pair 0 are projected first (kc-outer,
DMA-paced); then per token-tile the emission interleaves, under head h's
S^T/exp stream: the v projection (h0), the pair-1 q/k projections (h0),
and head h-1's attn@v matmuls (h1..h3), so TensorE work hides under the
exp stream and ScalarE never starves.  PSUM budget: 4 banks S^T ping-pong,
4 banks attn@v accumulators / step-specific projection accumulators.
"""

import sys

if "/opt/trn_rl_repo" not in sys.path:
    sys.path.insert(0, "/opt/trn_rl_repo")

import numpy as np
import ml_dtypes

import concourse.bacc as bacc
import concourse.tile as tile
import concourse.mybir as mybir
from concourse.bass_utils import run_bass_kernel_spmd

BF16 = mybir.dt.bfloat16
F32 = mybir.dt.float32
FP8 = mybir.dt.float8e4
NPBF16 = ml_dtypes.bfloat16

# fp8e4m3 E/v with DoubleRow matmuls for attn@v (2 fp8 weights per PE cell,
# contraction 256/pass).  exp(S) is ~1.0-scale so e4m3 is well-conditioned,
# and numerator/denominator share the same quantized E so the softmax ratio
# error largely cancels.
USE_FP8_AV = False
VW = 80  # per-head v columns incl. ones col, padded to a 16-byte stride

DIM = 1024
HEADS = 16
HEAD_DIM = 64
B, N = 2, 2048
SCALE = 1.0 / float(np.sqrt(np.float32(DIM)))

N_CORES = 8
GROUPS = 4             # head-groups (one per core within a batch)
HPG = HEADS // GROUPS  # heads per group = 4
DG = HPG * HEAD_DIM    # feature columns per group = 256

KC = DIM // 128        # contraction chunks over features = 8
MT = N // 128          # token tiles = 16
NB = N // 512          # 512-wide column blocks = 4
FT = DIM // 128        # output-feature tiles = 8
EXPW = 1024            # exp granularity (PSUM cols per S^T tile)
NH = N // EXPW         # halves per row-tile = 2


def build_kernel(reps_loop=False):
    """Build the per-core Bass program (identical on all cores; data differs).

    reps_loop=True wraps the body in a data-driven repeat loop (input tensor
    "reps") used only by the timing harness; the graded path builds without.
    """
    nc = bacc.Bacc("TRN2", target_bir_lowering=False, debug=False,
                   num_devices=N_CORES)

    xT = nc.dram_tensor("xT", [DIM, N], BF16, kind="ExternalInput")
    wqT = nc.dram_tensor("wqT", [DIM, DG], BF16, kind="ExternalInput")
    wkT = nc.dram_tensor("wkT", [DIM, DG], BF16, kind="ExternalInput")
    wvT = nc.dram_tensor("wvT", [DIM, DG], BF16, kind="ExternalInput")
    woT = nc.dram_tensor("woT", [DG, DIM], BF16, kind="ExternalInput")
    # q/k biases as per-pair columns [128, 2] f32 (partition = within-pair dim)
    bqc = nc.dram_tensor("bqc", [128, 2], F32, kind="ExternalInput")
    bkc = nc.dram_tensor("bkc", [128, 2], F32, kind="ExternalInput")
    bv = nc.dram_tensor("bv", [1, DG], BF16, kind="ExternalInput")
    outT = nc.dram_tensor("outT", [DIM, N], BF16, kind="ExternalOutput")
    reps = (nc.dram_tensor("reps", [1, 1], mybir.dt.int32,
                           kind="ExternalInput") if reps_loop else None)

    with tile.TileContext(nc) as tc:
        if reps_loop:
            with tc.tile_pool(name="repsp", bufs=1) as rpool:
                rt = rpool.tile([1, 1], mybir.dt.int32, tag="reps",
                                name="repst")
                nc.sync.dma_start(out=rt[:], in_=reps.ap()[:, :])
                val = nc.sync.value_load(rt[0:1, 0:1], min_val=1,
                                         max_val=1 << 20)
                with tc.For_i(0, val, 1):
                    _body(nc, tc, xT, wqT, wkT, wvT, woT, bqc, bkc, bv, outT)
        else:
            _body(nc, tc, xT, wqT, wkT, wvT, woT, bqc, bkc, bv, outT)

    nc.compile()
    return nc


def _body(nc, tc, xT, wqT, wkT, wvT, woT, bqc, bkc, bv, outT):
    from contextlib import ExitStack

    Exp = mybir.ActivationFunctionType.Exp

    with ExitStack() as ctx:
        persist = ctx.enter_context(tc.tile_pool(name="persist", bufs=1))
        e_pool = ctx.enter_context(tc.tile_pool(name="e_sb", bufs=36))
        sm_pool = ctx.enter_context(tc.tile_pool(name="attn_sm", bufs=8))
        xpool_cm = tc.tile_pool(name="xpool", bufs=1)
        xpool = xpool_cm.__enter__()

        # --- input loads: x/wq/wk interleaved per chunk (gates the ramp) ----
        xt_sb, wq_sb, wk_sb = [], [], []
        for kc in range(KC):
            t = xpool.tile([128, N], BF16, tag=f"xt{kc}", name=f"xt{kc}")
            nc.sync.dma_start(out=t[:], in_=xT.ap()[kc * 128:(kc + 1) * 128, :])
            xt_sb.append(t)
            t = xpool.tile([128, DG], BF16, tag=f"wq{kc}", name=f"wq{kc}")
            nc.sync.dma_start(out=t[:], in_=wqT.ap()[kc * 128:(kc + 1) * 128, :])
            wq_sb.append(t)
            t = xpool.tile([128, DG], BF16, tag=f"wk{kc}", name=f"wk{kc}")
            nc.sync.dma_start(out=t[:], in_=wkT.ap()[kc * 128:(kc + 1) * 128, :])
            wk_sb.append(t)
        bq_sb = persist.tile([128, 2], F32, tag="bq", name="bq")
        nc.sync.dma_start(out=bq_sb[:], in_=bqc.ap()[:, :])
        bk_sb = persist.tile([128, 2], F32, tag="bk", name="bk")
        nc.sync.dma_start(out=bk_sb[:], in_=bkc.ap()[:, :])
        wv_sb = []
        for kc in range(KC):
            t = xpool.tile([128, DG], BF16, tag=f"wv{kc}", name=f"wv{kc}")
            nc.sync.dma_start(out=t[:], in_=wvT.ap()[kc * 128:(kc + 1) * 128, :])
            wv_sb.append(t)
        bv_sb = xpool.tile([1, DG], BF16, tag="bv", name="bv")
        nc.sync.dma_start(out=bv_sb[:], in_=bv.ap()[:, :])
        wo_sb = []
        for pc in range(2):
            t = persist.tile([128, DIM], BF16, tag=f"wo{pc}", name=f"wo{pc}")
            nc.sync.dma_start(out=t[:], in_=woT.ap()[pc * 128:(pc + 1) * 128, :])
            wo_sb.append(t)
        ones = persist.tile([1, 512], BF16, tag="ones", name="ones")
        nc.vector.memset(ones[:], 1.0)
        # warm the ScalarE Exp table while DMAs stream in
        warm = persist.tile([1, 1], F32, tag="warm", name="warm")
        nc.scalar.activation(warm[:], ones[:, 0:1], Exp)

        # persistent activations
        qT_sb = [persist.tile([128, N], BF16, tag=f"qT{p}", name=f"qT{p}")
                 for p in range(2)]
        kT_sb = [persist.tile([128, N], BF16, tag=f"kT{p}", name=f"kT{p}")
                 for p in range(2)]
        if USE_FP8_AV:
            # paired token-tiles for DoubleRow: [128, (2, HPG, VW)] fp8
            v_sb = [persist.tile([128, 2 * HPG * VW], FP8, tag=f"v{mp}",
                                 name=f"v{mp}") for mp in range(MT // 2)]
        else:
            v_sb = [persist.tile([128, HPG * 65], BF16, tag=f"v{mt}",
                                 name=f"v{mt}") for mt in range(MT)]
        o_sb = [persist.tile([128, N], BF16, tag=f"oT{p}", name=f"oT{p}")
                for p in range(2)]

        # --- phase 1: q/k projections for pair 0, kc-outer (DMA-paced) -----
        with tc.tile_pool(name="qk0_ps", bufs=1, space="PSUM") as qk0:
            qacc = [qk0.tile([128, 512], F32, tag=f"qacc{nb}",
                             name=f"qacc{nb}") for nb in range(NB)]
            kacc = [qk0.tile([128, 512], F32, tag=f"kacc{nb}",
                             name=f"kacc{nb}") for nb in range(NB)]
            for kc in range(KC):
                for nb in range(NB):
                    nc.tensor.matmul(
                        qacc[nb][:],
                        lhsT=wq_sb[kc][:, 0:128],
                        rhs=xt_sb[kc][:, nb * 512:(nb + 1) * 512],
                        start=(kc == 0), stop=(kc == KC - 1))
                    nc.tensor.matmul(
                        kacc[nb][:],
                        lhsT=wk_sb[kc][:, 0:128],
                        rhs=xt_sb[kc][:, nb * 512:(nb + 1) * 512],
                        start=(kc == 0), stop=(kc == KC - 1))
            Ident = mybir.ActivationFunctionType.Identity
            for i, (which, nb) in enumerate(
                    (("q", 0), ("k", 0), ("k", 1), ("q", 1),
                     ("k", 2), ("k", 3), ("q", 2), ("q", 3))):
                acc, dst, bias = ((qacc, qT_sb, bq_sb) if which == "q"
                                  else (kacc, kT_sb, bk_sb))
                if i % 2 == 0:
                    nc.vector.tensor_scalar_add(
                        dst[0][:, nb * 512:(nb + 1) * 512], acc[nb][:],
                        bias[:, 0:1])
                else:
                    nc.scalar.activation(
                        dst[0][:, nb * 512:(nb + 1) * 512], acc[nb][:],
                        Ident, bias=bias[:, 0:1])

        # --- attention pipeline ---------------------------------------------
        s_pool_cm = tc.tile_pool(name="s_ps", bufs=2, space="PSUM")
        s_pool = s_pool_cm.__enter__()

        e_tiles = {}   # (h, mt, half) -> tile
        o_ps = {}      # h -> [4 psum accumulators]

        def emit_s_exp(h, mt):
            """S^T tile + exp for (head, token-tile), NH halves."""
            p, hh = divmod(h, 2)
            qs = qT_sb[p][hh * 64:(hh + 1) * 64, :]
            ks = kT_sb[p][hh * 64:(hh + 1) * 64, :]
            for half in range(NH):
                s_ps = s_pool.tile([128, EXPW], F32, tag="sps", name="sps")
                for j in range(EXPW // 512):
                    c0 = half * EXPW + j * 512
                    nc.tensor.matmul(
                        s_ps[:, j * 512:(j + 1) * 512],
                        lhsT=qs[:, mt * 128:(mt + 1) * 128],
                        rhs=ks[:, c0:c0 + 512],
                        start=True, stop=True)
                if USE_FP8_AV:
                    if mt % 2 == 0:
                        e_tiles[h, mt // 2, half] = e_pool.tile(
                            [128, 2 * EXPW], FP8, tag="e", name="e")
                    ep = e_tiles[h, mt // 2, half]
                    dst = ep.rearrange("p (two n) -> p two n",
                                       two=2)[:, mt % 2]
                    nc.scalar.activation(dst, s_ps[:], Exp, scale=SCALE)
                else:
                    e = e_pool.tile([128, EXPW], BF16, tag="e", name="e")
                    nc.scalar.activation(e[:], s_ps[:], Exp, scale=SCALE)
                    e_tiles[h, mt, half] = e

        def emit_av(h, mc, o_pool):
            """attn@[v|1] accumulation step for head h, m-chunk mc.

            fp8 path: mc indexes 256-row DoubleRow chunks (0..MT//2-1);
            bf16 path: mc indexes 128-row chunks (0..MT-1).
            """
            if mc == 0:
                o_ps[h] = [o_pool.tile([65, 512], F32, tag="ops",
                                       name="ops") for _ in range(NB)]
            if USE_FP8_AV:
                va = v_sb[mc].rearrange("p (two h c) -> p two h c",
                                        two=2, c=VW)[:, :, h, 0:65]
                for nb in range(NB):
                    ep = e_tiles[h, mc, nb // 2].rearrange(
                        "p (two n) -> p two n", two=2)
                    nc.tensor.matmul(
                        o_ps[h][nb][:],
                        lhsT=va,
                        rhs=ep[:, :, (nb % 2) * 512:(nb % 2 + 1) * 512],
                        start=(mc == 0), stop=(mc == MT // 2 - 1),
                        perf_mode=mybir.MatmulPerfMode.DoubleRow)
            else:
                va = v_sb[mc].rearrange("p (h c) -> p h c", c=65)[:, h, :]
                for nb in range(NB):
                    e = e_tiles[h, mc, nb // 2]
                    nc.tensor.matmul(
                        o_ps[h][nb][:],
                        lhsT=va,
                        rhs=e[:, (nb % 2) * 512:(nb % 2 + 1) * 512],
                        start=(mc == 0), stop=(mc == MT - 1))

        def emit_norm(h, nbs=None):
            """normalize O^T rows by the folded column-sums.

            Stage-major emission (recips, then broadcasts, then multiplies)
            so the three engines pipeline across the column blocks.
            """
            p, hh = divmod(h, 2)
            nbs = list(range(NB) if nbs is None else nbs)
            rs, bcs = {}, {}
            for nb in nbs:
                rs[nb] = sm_pool.tile([1, 512], F32, tag="recip",
                                      name="recip")
                nc.vector.reciprocal(rs[nb][:], o_ps[h][nb][64:65, :])
            for nb in nbs:
                bcs[nb] = sm_pool.tile([64, 512], F32, tag="bcast",
                                       name="bcast")
                nc.gpsimd.partition_broadcast(bcs[nb][:], rs[nb][:])
            for nb in nbs:
                nc.vector.tensor_mul(
                    o_sb[p][hh * 64:(hh + 1) * 64, nb * 512:(nb + 1) * 512],
                    o_ps[h][nb][0:64, :], bcs[nb][:])
            if nbs is None or list(nbs)[-1] == NB - 1:
                for key in [k for k in e_tiles if k[0] == h]:
                    del e_tiles[key]

        # --- step 2: head 0 S/exp + v projection + pair-1 q/k projections ---
        vps_cm = tc.tile_pool(name="vps", bufs=2, space="PSUM")
        vps = vps_cm.__enter__()
        p1ps_cm = tc.tile_pool(name="p1ps", bufs=2, space="PSUM")
        p1ps = p1ps_cm.__enter__()

        def emit_v(mt):
            ps = vps.tile([128, DG], F32, tag="vps", name="vpsn")
            for kc in range(KC):
                nc.tensor.matmul(
                    ps[:],
                    lhsT=xt_sb[kc][:, mt * 128:(mt + 1) * 128],
                    rhs=wv_sb[kc][:],
                    start=(kc == 0), stop=False)
            nc.tensor.matmul(
                ps[:], lhsT=ones[:, :128], rhs=bv_sb[:],
                start=False, stop=True)
            if USE_FP8_AV:
                dst = v_sb[mt // 2].rearrange(
                    "p (two h c) -> p two h c", two=2, c=VW)[:, mt % 2]
            else:
                dst = v_sb[mt].rearrange("p (h c) -> p h c", c=65)
            nc.vector.tensor_copy(dst[:, :, 0:64],
                                  ps.rearrange("p (h c) -> p h c", c=64))
            nc.vector.memset(dst[:, :, 64:65], 1.0)

        def emit_p1_group(i):
            """one (name, nb) accumulation group of the pair-1 projections."""
            name, nb = divmod(i, NB)
            w, bias, dst = ((wq_sb, bq_sb, qT_sb) if name == 0
                            else (wk_sb, bk_sb, kT_sb))
            ps = p1ps.tile([128, 512], F32, tag="p1", name="p1")
            for kc in range(KC):
                nc.tensor.matmul(
                    ps[:],
                    lhsT=w[kc][:, 128:256],
                    rhs=xt_sb[kc][:, nb * 512:(nb + 1) * 512],
                    start=(kc == 0), stop=(kc == KC - 1))
            nc.vector.tensor_scalar_add(
                dst[1][:, nb * 512:(nb + 1) * 512], ps[:], bias[:, 1:2])

        for mt in range(MT):
            emit_v(mt)
            emit_s_exp(0, mt)
            if mt % 2 == 1:
                emit_p1_group(mt // 2)

        p1ps_cm.__exit__(None, None, None)
        vps_cm.__exit__(None, None, None)

        o_pool_cm = tc.tile_pool(name="o_ps", bufs=4, space="PSUM")
        o_pool = o_pool_cm.__enter__()

        # --- steps 3-4: heads 1-2 S/exp + previous head's attn@v ------------
        for h in (1, 2):
            for mt in range(MT):
                emit_s_exp(h, mt)
                if USE_FP8_AV:
                    if mt % 2 == 1:
                        emit_av(h - 1, mt // 2, o_pool)
                else:
                    emit_av(h - 1, mt, o_pool)
            emit_norm(h - 1)

        # --- step 5: head 3 S/exp + attn@v of heads 2 and 3 -----------------
        for mt in range(MT):
            emit_s_exp(3, mt)
            if USE_FP8_AV:
                if mt < 8:
                    emit_av(2, mt, o_pool)
                    if mt == 7:
                        emit_norm(2)
                else:
                    emit_av(3, mt - 8, o_pool)
            else:
                if mt < 8:
                    emit_av(2, 2 * mt, o_pool)
                    emit_av(2, 2 * mt + 1, o_pool)
                    if mt == 7:
                        emit_norm(2)
                else:
                    emit_av(3, 2 * (mt - 8), o_pool)
                    emit_av(3, 2 * (mt - 8) + 1, o_pool)
        emit_norm(3)

        o_pool_cm.__exit__(None, None, None)
        s_pool_cm.__exit__(None, None, None)
        xpool_cm.__exit__(None, None, None)

        # --- output projection (partial: this group's rows of Wo.T) ---------
        # nb-outer so norm(3, nb) -> matmuls -> drains -> DMA pipeline per
        # column block; output in bf16 to halve the tail DMA.
        with (
            tc.tile_pool(name="out_ps", bufs=8, space="PSUM") as out_pool,
            tc.tile_pool(name="out_sb", bufs=8) as ostage,
        ):
            for nb in range(NB):
                for ft in range(FT):
                    ps = out_pool.tile([128, 512], F32, tag="outps",
                                       name="outps")
                    for pc in range(2):
                        nc.tensor.matmul(
                            ps[:],
                            lhsT=wo_sb[pc][:, ft * 128:(ft + 1) * 128],
                            rhs=o_sb[pc][:, nb * 512:(nb + 1) * 512],
                            start=(pc == 0), stop=(pc == 1))
                    stage = ostage.tile([128, 512], BF16, tag="ostage",
                                        name="ostage")
                    # both ScalarE and VectorE are idle by now; split drains
                    if ft % 2 == 0:
                        nc.scalar.copy(stage[:], ps[:])
                    else:
                        nc.vector.tensor_copy(stage[:], ps[:])
                    nc.sync.dma_start(
                        out=outT.ap()[ft * 128:(ft + 1) * 128,
                                      nb * 512:(nb + 1) * 512],
                        in_=stage[:])


_CACHED_NC = None


def _get_nc():
    global _CACHED_NC
    if _CACHED_NC is None:
        _CACHED_NC = build_kernel()
    return _CACHED_NC


def make_in_maps(x, Wq, bq, Wk, bk, Wv, bv, Wo, bo):
    """Host-side shard/layout prep: per-core input dict."""
    x = np.asarray(x, dtype=np.float32)
    xT_b = [np.ascontiguousarray(x[b].T).astype(NPBF16) for b in range(B)]
    WqT = np.asarray(Wq, np.float32).T.astype(NPBF16)  # [DIM(feat), DIM(out)]
    WkT = np.asarray(Wk, np.float32).T.astype(NPBF16)
    WvT = np.asarray(Wv, np.float32).T.astype(NPBF16)
    WoT = np.asarray(Wo, np.float32).T.astype(NPBF16)  # rows: concat feats
    bq = np.asarray(bq, np.float32)
    bk = np.asarray(bk, np.float32)
    bv16 = np.asarray(bv, np.float32).astype(NPBF16)

    in_maps = []
    for c in range(N_CORES):
        b, g = divmod(c, GROUPS)
        sl = slice(g * DG, (g + 1) * DG)
        in_maps.append({
            "xT": xT_b[b],
            "wqT": np.ascontiguousarray(WqT[:, sl]),
            "wkT": np.ascontiguousarray(WkT[:, sl]),
            "wvT": np.ascontiguousarray(WvT[:, sl]),
            "woT": np.ascontiguousarray(WoT[sl, :]),
            "bqc": np.ascontiguousarray(bq[sl].reshape(2, 128).T),
            "bkc": np.ascontiguousarray(bk[sl].reshape(2, 128).T),
            "bv": bv16[sl].reshape(1, DG),
        })
    return in_maps


def combine_outputs(results, bo):
    """Host-side unshard: sum group partials per batch, add bo."""
    bo = np.asarray(bo, np.float32)
    out = np.zeros((B, N, DIM), np.float32)
    for c in range(N_CORES):
        b = c // GROUPS
        out[b] += results[c]["outT"].astype(np.float32).T
    out += bo
    return out


def kernel(**inputs):
    nc = _get_nc()
    in_maps = make_in_maps(**{k: inputs[k] for k in
                              ("x", "Wq", "bq", "Wk", "bk", "Wv", "bv",
                               "Wo", "bo")})
    res = run_bass_kernel_spmd(nc, in_maps, list(range(N_CORES)))
    return combine_outputs(res.results, inputs["bo"])


if __name__ == "__main__":
    rng = np.random.default_rng(0)
    ins = {
        "x": rng.standard_normal((B, N, DIM), np.float32),
        "Wq": rng.standard_normal((DIM, DIM), np.float32) * 0.02,
        "bq": rng.standard_normal((DIM,), np.float32) * 0.02,
        "bk": rng.standard_normal((DIM,), np.float32) * 0.02,
        "Wk": rng.standard_normal((DIM, DIM), np.float32) * 0.02,
        "Wv": rng.standard_normal((DIM, DIM), np.float32) * 0.02,
        "bv": rng.standard_normal((DIM,), np.float32) * 0.02,
        "Wo": rng.standard_normal((DIM, DIM), np.float32) * 0.02,
        "bo": rng.standard_normal((DIM,), np.float32) * 0.02,
    }
    out = kernel(**ins)
    print("kernel output", out.shape, out.dtype, float(np.abs(out).mean()))



# revision 7
# speedup vs baseline: 1.1039x; 1.1039x over previous
"""Trainium2 Bass kernel for nn_MultiHeadAttention_5059471475068.

Reference computation (B=2, N=2048, DIM=1024, H=16 heads, d=64):
    q = x @ Wq.T + bq ; k = x @ Wk.T + bk ; v = x @ Wv.T + bv   (per-head split)
    scores[h,b,n,m] = (k[h,b,n,:] . q[h,b,m,:]) / sqrt(DIM)
    attn = softmax(scores, axis=m)
    out[h,b,n,:] = attn @ v ; out = concat_heads @ Wo.T + bo

Sharding: 8 cores = 2 batches x 4 head-groups (4 heads per core).  Host sums
the 4 partial output projections per batch and adds bo.

Per-core structure (all cost figures are TimelineSim/TRN2 model):
  * q/k projections run as fp8e4+DoubleRow matmuls (x and 32*W quantized to
    fp8, contraction pairs packed in the free dim), writing q',k' = 32*(q,k)
    straight back to fp8 SBUF in the DR pair layout the scores matmuls want.
  * scores S'[m,n] = q'_m . k'_n are fp8+DoubleRow with d=64 packed as 32
    partitions x 2.  exp scale absorbs the 32*32 factor (2^-15).
  * softmax numerators: most tiles exact Exp on ScalarE; a fixed subset uses
    the Taylor factorization e^S ~ (1+S/2)^2 computed as one DVE
    tensor_scalar (t = S*c + 1, PSUM read) plus one GpSimd square
    (e = t*t, SBUF only), keeping ScalarE off the critical path.
  * attn@v keeps E tiles **stationary** ([128 m x 128 n] chunks) and streams
    [v | 1] (65 cols) as the moving operand, so the narrow per-head v width
    costs moving-cycles instead of wasting stationary width.  PSUM row
    accumulators live as 65-col slices of three bank tiles; col 64
    accumulates the softmax denominator.
  * normalization is a per-partition tensor_scalar multiply (tokens are on
    partitions after the restructured attn@v), then a PE transpose brings
    o back to [d, n] for the bf16 output projection.
"""

import sys

if "/opt/trn_rl_repo" not in sys.path:
    sys.path.insert(0, "/opt/trn_rl_repo")

import numpy as np
import ml_dtypes

import concourse.bacc as bacc
import concourse.tile as tile
import concourse.mybir as mybir
from concourse.bass_utils import run_bass_kernel_spmd

BF16 = mybir.dt.bfloat16
F32 = mybir.dt.float32
FP8 = mybir.dt.float8e4
NPBF16 = ml_dtypes.bfloat16
NPFP8 = ml_dtypes.float8_e4m3

DIM = 1024
HEADS = 16
HEAD_DIM = 64
B, N = 2, 2048

N_CORES = 8
GROUPS = 4             # head-groups (one per core within a batch)
HPG = HEADS // GROUPS  # heads per group = 4
DG = HPG * HEAD_DIM    # feature columns per group = 256

WS = 32.0                       # fp8 weight pre-scale for q/k projections
SCALE_EFF = float(2.0 ** -15)   # (1/sqrt(1024)) / (WS*WS)
TAYC = float(2.0 ** -16)        # SCALE_EFF/2 for the (1+S/2)^2 tiles

XC = 4                 # x fp8 chunks (256 features each, DR pairs)
MT = N // 128          # token tiles = 16
NB = N // 512          # 512-wide column blocks = 4
FT = DIM // 128        # output-feature tiles = 8
AV_LAG = 3             # attn@v trails exp by this many m-tiles

Mult = mybir.AluOpType.mult
Add = mybir.AluOpType.add
DR = mybir.MatmulPerfMode.DoubleRow


def _unit_engine(h, mt, half):
    """softmax tile -> engine: 'act' (exact exp) or 'pool' (DVE ts + gpsimd
    square Taylor).  ~1/3 of tiles go to the DVE+Pool pair."""
    uid = h * 32 + mt * 2 + half
    if False:
        return "pool"
    return "act"


def build_kernel():
    nc = bacc.Bacc("TRN2", target_bir_lowering=False, debug=False,
                   num_devices=N_CORES)

    xT = nc.dram_tensor("xT", [DIM, N], BF16, kind="ExternalInput")
    x8 = nc.dram_tensor("x8", [XC * 128, 2 * N], FP8, kind="ExternalInput")
    wq8 = nc.dram_tensor("wq8", [XC * 128, 512], FP8, kind="ExternalInput")
    wk8 = nc.dram_tensor("wk8", [XC * 128, 512], FP8, kind="ExternalInput")
    bqc = nc.dram_tensor("bqc", [128, 2], F32, kind="ExternalInput")
    bkc = nc.dram_tensor("bkc", [128, 2], F32, kind="ExternalInput")
    wvT = nc.dram_tensor("wvT", [DIM, DG], BF16, kind="ExternalInput")
    bv = nc.dram_tensor("bv", [1, DG], BF16, kind="ExternalInput")
    woT = nc.dram_tensor("woT", [DG, DIM], BF16, kind="ExternalInput")
    ident = nc.dram_tensor("ident", [128, 128], BF16, kind="ExternalInput")
    outT = nc.dram_tensor("outT", [DIM, N], BF16, kind="ExternalOutput")

    with tile.TileContext(nc) as tc:
        _body(nc, tc, xT, x8, wq8, wk8, bqc, bkc, wvT, bv, woT, ident, outT)

    nc.compile()
    return nc


def _body(nc, tc, xT, x8, wq8, wk8, bqc, bkc, wvT, bv, woT, ident, outT):
    from contextlib import ExitStack

    Exp = mybir.ActivationFunctionType.Exp

    with ExitStack() as ctx:
        persist = ctx.enter_context(tc.tile_pool(name="persist", bufs=1))
        e_pool = ctx.enter_context(tc.tile_pool(name="e_sb", bufs=10))
        t_pool = ctx.enter_context(tc.tile_pool(name="t_sb", bufs=3))
        on_pool = ctx.enter_context(tc.tile_pool(name="on_sb", bufs=18))

        # --- input loads ----------------------------------------------------
        x8_sb, wq8_sb, wk8_sb = [], [], []
        for c in range(XC):
            t = persist.tile([128, 2 * N], FP8, tag=f"x8{c}", name=f"x8{c}")
            nc.sync.dma_start(out=t[:], in_=x8.ap()[c * 128:(c + 1) * 128, :])
            x8_sb.append(t.rearrange("p (j n) -> p j n", j=2))
            t = persist.tile([128, 512], FP8, tag=f"wq8{c}", name=f"wq8{c}")
            nc.sync.dma_start(out=t[:], in_=wq8.ap()[c * 128:(c + 1) * 128, :])
            wq8_sb.append(t.rearrange("p (j ji c) -> p j ji c", j=2, ji=2))
            t = persist.tile([128, 512], FP8, tag=f"wk8{c}", name=f"wk8{c}")
            nc.sync.dma_start(out=t[:], in_=wk8.ap()[c * 128:(c + 1) * 128, :])
            wk8_sb.append(t.rearrange("p (j ji c) -> p j ji c", j=2, ji=2))
        bq_sb = persist.tile([128, 2], F32, tag="bq", name="bq")
        nc.sync.dma_start(out=bq_sb[:], in_=bqc.ap()[:, :])
        bk_sb = persist.tile([128, 2], F32, tag="bk", name="bk")
        nc.sync.dma_start(out=bk_sb[:], in_=bkc.ap()[:, :])
        id_sb = persist.tile([128, 128], BF16, tag="ident", name="ident")
        nc.sync.dma_start(out=id_sb[:], in_=ident.ap()[:, :])
        xt_sb = []
        for kc in range(8):
            t = persist.tile([128, N], BF16, tag=f"xt{kc}", name=f"xt{kc}")
            nc.sync.dma_start(out=t[:], in_=xT.ap()[kc * 128:(kc + 1) * 128, :])
            xt_sb.append(t)
        wv_sb = []
        for kc in range(8):
            t = persist.tile([128, DG], BF16, tag=f"wv{kc}", name=f"wv{kc}")
            nc.sync.dma_start(out=t[:], in_=wvT.ap()[kc * 128:(kc + 1) * 128, :])
            wv_sb.append(t)
        bv_sb = persist.tile([1, DG], BF16, tag="bv", name="bv")
        nc.sync.dma_start(out=bv_sb[:], in_=bv.ap()[:, :])
        wo_sb = []
        for pc in range(2):
            t = persist.tile([128, DIM], BF16, tag=f"wo{pc}", name=f"wo{pc}")
            nc.sync.dma_start(out=t[:], in_=woT.ap()[pc * 128:(pc + 1) * 128, :])
            wo_sb.append(t)
        ones = persist.tile([1, 512], BF16, tag="ones", name="ones")
        nc.vector.memset(ones[:], 1.0)
        # warm the ScalarE Exp table while DMAs stream in
        warm = persist.tile([1, 1], F32, tag="warm", name="warm")
        nc.scalar.activation(warm[:], ones[:, 0:1], Exp)

        # persistent activations
        qt = persist.tile([128, 2 * N], FP8, tag="qt", name="qt")
        kt = persist.tile([128, 2 * N], FP8, tag="kt", name="kt")
        qt_r = qt.rearrange("p (j n) -> p j n", j=2)
        kt_r = kt.rearrange("p (j n) -> p j n", j=2)
        v_sb = [persist.tile([128, HPG * 65], BF16, tag=f"v{mt}",
                             name=f"v{mt}") for mt in range(MT)]
        oT_sb = [persist.tile([128, N], BF16, tag=f"oT{p}", name=f"oT{p}")
                 for p in range(2)]
        recip = persist.tile([128, 16], F32, tag="recip", name="recip")

        # --- phase 1: q/k projections (fp8 DoubleRow) -----------------------
        # PSUM pass (tensor, jpass, nb) accumulates 4 DR chunk-matmuls; DVE
        # evicts with the per-partition bias straight into the fp8 pair
        # layout.  Order feeds kt for n-halves before qt token blocks.
        with tc.tile_pool(name="qk_ps", bufs=4, space="PSUM") as qkp:
            def proj_group(w_sb, bias_sb, dst_r, j, nb):
                ps = qkp.tile([128, 512], F32, tag="qkps", name="qkps")
                for c in range(XC):
                    nc.tensor.matmul(
                        ps[:],
                        lhsT=w_sb[c][:, j],
                        rhs=x8_sb[c][:, :, nb * 512:(nb + 1) * 512],
                        start=(c == 0), stop=(c == XC - 1), perf_mode=DR)
                nc.vector.tensor_scalar(
                    dst_r[:, j, nb * 512:(nb + 1) * 512], ps[:],
                    bias_sb[:, j:j + 1], None, Add)

            for nb in range(NB):
                for j in range(2):
                    proj_group(wk8_sb, bk_sb, kt_r, j, nb)
                for j in range(2):
                    proj_group(wq8_sb, bq_sb, qt_r, j, nb)

        # --- main pipeline ---------------------------------------------------
        s_pool_cm = tc.tile_pool(name="s_ps", bufs=2, space="PSUM")
        s_pool = s_pool_cm.__enter__()
        av_cm = tc.tile_pool(name="av_ps", bufs=1, space="PSUM")
        avp = av_cm.__enter__()
        avA = avp.tile([128, 455], F32, tag="avA", name="avA")
        avB = avp.tile([128, 455], F32, tag="avB", name="avB")
        avC = avp.tile([128, 130], F32, tag="avC", name="avC")
        avA_r = avA.rearrange("p (nb c) -> p nb c", c=65)
        avB_r = avB.rearrange("p (nb c) -> p nb c", c=65)
        avC_r = avC.rearrange("p (nb c) -> p nb c", c=65)

        def av_slice(nb):
            if nb < 7:
                return avA[:, nb * 65:(nb + 1) * 65]
            if nb < 14:
                return avB[:, (nb - 7) * 65:(nb - 6) * 65]
            return avC[:, (nb - 14) * 65:(nb - 13) * 65]

        e_tiles = {}

        def emit_s_exp(h, mt, half):
            s_ps = s_pool.tile([128, 1024], F32, tag="sps", name="sps")
            for jj in range(2):
                c0 = half * 1024 + jj * 512
                nc.tensor.matmul(
                    s_ps[:, jj * 512:(jj + 1) * 512],
                    lhsT=qt_r[32 * h:32 * h + 32, :, mt * 128:(mt + 1) * 128],
                    rhs=kt_r[32 * h:32 * h + 32, :, c0:c0 + 512],
                    start=True, stop=True, perf_mode=DR,
                    tile_position=(32 * h, 0))
            e = e_pool.tile([128, 1024], BF16, tag="e", name="e")
            if _unit_engine(h, mt, half) == "act":
                nc.scalar.activation(e[:], s_ps[:], Exp, scale=SCALE_EFF)
            else:
                t = t_pool.tile([128, 1024], BF16, tag="t", name="t")
                nc.vector.tensor_scalar(t[:], s_ps[:], TAYC, 1.0, Mult, Add)
                nc.gpsimd.tensor_mul(e[:], t[:], t[:])
            e_tiles[h, mt, half] = e

        def emit_av(h, mc):
            v_r = v_sb[mc].rearrange("p (h c) -> p h c", c=65)
            for nb in range(16):
                e = e_tiles[h, mc, nb // 8]
                # start=True zeroes the whole PSUM bank, so only the first
                # slice per bank (nb 0/7/14) carries it; siblings accumulate
                # onto the bank-zeroed state.
                nc.tensor.matmul(
                    av_slice(nb),
                    lhsT=e[:, (nb % 8) * 128:(nb % 8 + 1) * 128],
                    rhs=v_r[:, h, :],
                    start=(mc == 0 and nb in (0, 7, 14)),
                    stop=(mc == MT - 1))
            if mc == MT - 1:
                for half in range(2):
                    del e_tiles[h, mc, half]
            elif mc >= 1:
                for half in range(2):
                    del e_tiles[h, mc - 1, half]

        def emit_v(vps, mt):
            ps = vps.tile([128, DG], F32, tag="vps", name="vpsn")
            for kc in range(8):
                nc.tensor.matmul(
                    ps[:],
                    lhsT=xt_sb[kc][:, mt * 128:(mt + 1) * 128],
                    rhs=wv_sb[kc][:],
                    start=(kc == 0), stop=False)
            nc.tensor.matmul(
                ps[:], lhsT=ones[:, :128], rhs=bv_sb[:],
                start=False, stop=True)
            dst = v_sb[mt].rearrange("p (h c) -> p h c", c=65)
            nc.vector.tensor_copy(dst[:, :, 0:64],
                                  ps.rearrange("p (h c) -> p h c", c=64))
            nc.vector.memset(dst[:, :, 64:65], 1.0)

        def emit_norm(h):
            nc.vector.reciprocal(recip[:, 0:7], avA_r[:, :, 64])
            nc.vector.reciprocal(recip[:, 7:14], avB_r[:, :, 64])
            nc.vector.reciprocal(recip[:, 14:16], avC_r[:, :, 64])
            o_n = []
            for nb in range(16):
                t = on_pool.tile([128, 64], BF16, tag="on", name="on")
                nc.vector.tensor_scalar(
                    t[:], av_slice(nb)[:, 0:64], recip[:, nb:nb + 1], None,
                    Mult)
                o_n.append(t)
            return o_n

        def emit_tr_group(tr_pool, h, o_n, g):
            """transpose o_n[4g..4g+4) and evict to oT (one 512-col group)."""
            trt = tr_pool.tile([64, 512], BF16, tag="tr", name="tr")
            for i in range(4):
                nc.tensor.transpose(
                    trt[:, i * 128:(i + 1) * 128], o_n[4 * g + i][:], id_sb[:])
            p, hh = divmod(h, 2)
            nc.vector.tensor_copy(
                oT_sb[p][hh * 64:(hh + 1) * 64, g * 512:(g + 1) * 512], trt[:])

        # ---- head 0 (with v projection, av lagged) -------------------------
        vps_cm = tc.tile_pool(name="vps", bufs=1, space="PSUM")
        vps = vps_cm.__enter__()
        for mt in range(MT):
            emit_s_exp(0, mt, 0)
            emit_v(vps, mt)
            emit_s_exp(0, mt, 1)
            if mt >= AV_LAG:
                emit_av(0, mt - AV_LAG)
        for mc in range(MT - AV_LAG, MT):
            emit_av(0, mc)
        vps_cm.__exit__(None, None, None)
        tr_cm = tc.tile_pool(name="tr_ps", bufs=1, space="PSUM")
        tr_pool = tr_cm.__enter__()
        pend = (0, emit_norm(0))

        # ---- heads 1..3 ----------------------------------------------------
        for h in (1, 2, 3):
            for mt in range(MT):
                emit_s_exp(h, mt, 0)
                emit_s_exp(h, mt, 1)
                if mt >= AV_LAG:
                    emit_av(h, mt - AV_LAG)
                if pend is not None and AV_LAG <= mt < AV_LAG + 4:
                    ph, po = pend
                    emit_tr_group(tr_pool, ph, po, mt - AV_LAG)
                    if mt == AV_LAG + 3:
                        pend = None
            for mc in range(MT - AV_LAG, MT):
                emit_av(h, mc)
            pend = (h, emit_norm(h))

        # head-3 transposes before the PSUM pools unwind (LIFO release)
        ph, po = pend
        for g in range(4):
            emit_tr_group(tr_pool, ph, po, g)
        tr_cm.__exit__(None, None, None)
        av_cm.__exit__(None, None, None)
        s_pool_cm.__exit__(None, None, None)

        # ---- tail: output projection ---------------------------------------
        with (
            tc.tile_pool(name="out_ps", bufs=4, space="PSUM") as out_pool,
            tc.tile_pool(name="out_sb", bufs=8) as ostage,
        ):
            for nb in range(NB):
                for ft in range(FT):
                    ps = out_pool.tile([128, 512], F32, tag="outps",
                                       name="outps")
                    for pc in range(2):
                        nc.tensor.matmul(
                            ps[:],
                            lhsT=wo_sb[pc][:, ft * 128:(ft + 1) * 128],
                            rhs=oT_sb[pc][:, nb * 512:(nb + 1) * 512],
                            start=(pc == 0), stop=(pc == 1))
                    stage = ostage.tile([128, 512], BF16, tag="ostage",
                                        name="ostage")
                    if (nb * FT + ft) % 3 == 2:
                        nc.vector.tensor_copy(stage[:], ps[:])
                    else:
                        nc.scalar.copy(stage[:], ps[:])
                    nc.sync.dma_start(
                        out=outT.ap()[ft * 128:(ft + 1) * 128,
                                      nb * 512:(nb + 1) * 512],
                        in_=stage[:])


_CACHED_NC = None


def _get_nc():
    global _CACHED_NC
    if _CACHED_NC is None:
        _CACHED_NC = build_kernel()
    return _CACHED_NC


def _pack_w8(WT_g, ws):
    """[1024, 256] slice of W.T (in-feat, out-feat) -> [512, 512] fp8 DR
    layout: row c*128+p_in, col jpass*256 + j_in*128 + (h*32 + r)."""
    A = (ws * WT_g).astype(np.float32)
    A3 = A.reshape(XC, 2, 128, HPG, 2, 32)      # c, j_in, p_in, h, jpass, r
    A3 = A3.transpose(0, 2, 4, 1, 3, 5)          # c, p_in, jpass, j_in, h, r
    return np.ascontiguousarray(A3.reshape(XC * 128, 512)).astype(NPFP8)


def _pack_b(b_g, ws):
    """[256] group bias -> [128, 2] f32: [p, jpass] = ws*b[64*(p//32) +
    jpass*32 + p%32]."""
    A = (ws * b_g).astype(np.float32).reshape(HPG, 2, 32)  # h, jpass, r
    return np.ascontiguousarray(A.transpose(0, 2, 1).reshape(128, 2))


def make_in_maps(x, Wq, bq, Wk, bk, Wv, bv, Wo, bo):
    """Host-side shard/layout prep: per-core input dict."""
    x = np.asarray(x, dtype=np.float32)
    xT_b = [np.ascontiguousarray(x[b].T) for b in range(B)]
    x8_b = []
    for b in range(B):
        xr = xT_b[b].reshape(XC, 2, 128, N)      # c, j, p, n
        xr = xr.transpose(0, 2, 1, 3)            # c, p, j, n
        x8_b.append(np.ascontiguousarray(xr.reshape(XC * 128, 2 * N))
                    .astype(NPFP8))
    WqT = np.asarray(Wq, np.float32).T  # [in-feat, out-feat]
    WkT = np.asarray(Wk, np.float32).T
    WvT = np.asarray(Wv, np.float32).T
    WoT = np.asarray(Wo, np.float32).T
    bq = np.asarray(bq, np.float32)
    bk = np.asarray(bk, np.float32)
    bv16 = np.asarray(bv, np.float32).astype(NPBF16)
    ident = np.eye(128, dtype=np.float32).astype(NPBF16)

    in_maps = []
    for c in range(N_CORES):
        b, g = divmod(c, GROUPS)
        sl = slice(g * DG, (g + 1) * DG)
        in_maps.append({
            "xT": xT_b[b].astype(NPBF16),
            "x8": x8_b[b],
            "wq8": _pack_w8(WqT[:, sl], WS),
            "wk8": _pack_w8(WkT[:, sl], WS),
            "bqc": _pack_b(bq[sl], WS),
            "bkc": _pack_b(bk[sl], WS),
            "wvT": np.ascontiguousarray(WvT[:, sl]).astype(NPBF16),
            "bv": bv16[sl].reshape(1, DG),
            "woT": np.ascontiguousarray(WoT[sl, :]).astype(NPBF16),
            "ident": ident,
        })
    return in_maps


def combine_outputs(results, bo):
    """Host-side unshard: sum group partials per batch, add bo."""
    bo = np.asarray(bo, np.float32)
    out = np.zeros((B, N, DIM), np.float32)
    for c in range(N_CORES):
        b = c // GROUPS
        out[b] += results[c]["outT"].astype(np.float32).T
    out += bo
    return out


def kernel(**inputs):
    nc = _get_nc()
    in_maps = make_in_maps(**{k: inputs[k] for k in
                              ("x", "Wq", "bq", "Wk", "bk", "Wv", "bv",
                               "Wo", "bo")})
    res = run_bass_kernel_spmd(nc, in_maps, list(range(N_CORES)))
    return combine_outputs(res.results, inputs["bo"])


if __name__ == "__main__":
    rng = np.random.default_rng(0)
    ins = {
        "x": rng.standard_normal((B, N, DIM), np.float32),
        "Wq": rng.standard_normal((DIM, DIM), np.float32) * 0.02,
        "bq": rng.standard_normal((DIM,), np.float32) * 0.02,
        "bk": rng.standard_normal((DIM,), np.float32) * 0.02,
        "Wk": rng.standard_normal((DIM, DIM), np.float32) * 0.02,
        "Wv": rng.standard_normal((DIM, DIM), np.float32) * 0.02,
        "bv": rng.standard_normal((DIM,), np.float32) * 0.02,
        "Wo": rng.standard_normal((DIM, DIM), np.float32) * 0.02,
        "bo": rng.standard_normal((DIM,), np.float32) * 0.02,
    }
    out = kernel(**ins)
    print("kernel output", out.shape, out.dtype, float(np.abs(out).mean()))


# revision 9
# speedup vs baseline: 1.1308x; 1.0243x over previous
"""Trainium2 Bass kernel for nn_MultiHeadAttention_5059471475068.

Reference computation (B=2, N=2048, DIM=1024, H=16 heads, d=64):
    q = x @ Wq.T + bq ; k = x @ Wk.T + bk ; v = x @ Wv.T + bv   (per-head split)
    scores[h,b,n,m] = (k[h,b,n,:] . q[h,b,m,:]) / sqrt(DIM)
    attn = softmax(scores, axis=m)
    out[h,b,n,:] = attn @ v ; out = concat_heads @ Wo.T + bo

Sharding: 8 cores = 2 batches x 4 head-groups (4 heads per core).  Host sums
the 4 partial output projections per batch and adds bo.

Per-core structure (all cost figures are TimelineSim/TRN2 model):
  * q/k projections run as fp8e4+DoubleRow matmuls (x and 32*W quantized to
    fp8, contraction pairs packed in the free dim), writing q',k' = 32*(q,k)
    straight back to fp8 SBUF in the DR pair layout the scores matmuls want.
  * scores S'[m,n] = q'_m . k'_n are fp8+DoubleRow with d=64 packed as 32
    partitions x 2.  exp scale absorbs the 32*32 factor (2^-15).
  * softmax numerators: most tiles exact Exp on ScalarE; a fixed subset uses
    the Taylor factorization e^S ~ (1+S/2)^2 computed as one DVE
    tensor_scalar (t = S*c + 1, PSUM read) plus one GpSimd square
    (e = t*t, SBUF only), keeping ScalarE off the critical path.
  * attn@v keeps E tiles **stationary** ([128 m x 128 n] chunks) and streams
    [v | 1] (65 cols) as the moving operand, so the narrow per-head v width
    costs moving-cycles instead of wasting stationary width.  PSUM row
    accumulators live as 65-col slices of three bank tiles; col 64
    accumulates the softmax denominator.
  * normalization is a per-partition tensor_scalar multiply (tokens are on
    partitions after the restructured attn@v), then a PE transpose brings
    o back to [d, n] for the bf16 output projection.
"""

import sys

if "/opt/trn_rl_repo" not in sys.path:
    sys.path.insert(0, "/opt/trn_rl_repo")

import numpy as np
import ml_dtypes

import concourse.bacc as bacc
import concourse.tile as tile
import concourse.mybir as mybir
from concourse.bass_utils import run_bass_kernel_spmd

BF16 = mybir.dt.bfloat16
F32 = mybir.dt.float32
FP8 = mybir.dt.float8e4
NPBF16 = ml_dtypes.bfloat16
NPFP8 = ml_dtypes.float8_e4m3

DIM = 1024
HEADS = 16
HEAD_DIM = 64
B, N = 2, 2048

N_CORES = 8
GROUPS = 4             # head-groups (one per core within a batch)
HPG = HEADS // GROUPS  # heads per group = 4
DG = HPG * HEAD_DIM    # feature columns per group = 256

WS = 32.0                       # fp8 weight pre-scale for q/k projections
SCALE_EFF = float(2.0 ** -15)   # (1/sqrt(1024)) / (WS*WS)
TAYC = float(2.0 ** -16)        # SCALE_EFF/2 for the (1+S/2)^2 tiles

XC = 4                 # x fp8 chunks (256 features each, DR pairs)
MT = N // 128          # token tiles = 16
NB = N // 512          # 512-wide column blocks = 4
FT = DIM // 128        # output-feature tiles = 8
AV_LAG = 3             # attn@v trails exp by this many m-tiles

Mult = mybir.AluOpType.mult
Add = mybir.AluOpType.add
DR = mybir.MatmulPerfMode.DoubleRow


def _unit_engine(h, mt, half):
    """softmax tile -> engine: 'act' (exact exp) or 'pool' (DVE ts + gpsimd
    square Taylor).  ~1/3 of tiles go to the DVE+Pool pair."""
    uid = h * 32 + mt * 2 + half
    if False:
        return "pool"
    return "act"


def build_kernel():
    nc = bacc.Bacc("TRN2", target_bir_lowering=False, debug=False,
                   num_devices=N_CORES)

    xT = nc.dram_tensor("xT", [DIM, N], BF16, kind="ExternalInput")
    x8 = nc.dram_tensor("x8", [XC * 128, 2 * N], FP8, kind="ExternalInput")
    wq8 = nc.dram_tensor("wq8", [XC * 128, 512], FP8, kind="ExternalInput")
    wk8 = nc.dram_tensor("wk8", [XC * 128, 512], FP8, kind="ExternalInput")
    bqc = nc.dram_tensor("bqc", [128, 2], F32, kind="ExternalInput")
    bkc = nc.dram_tensor("bkc", [128, 2], F32, kind="ExternalInput")
    wvT = nc.dram_tensor("wvT", [DIM, DG], BF16, kind="ExternalInput")
    bv = nc.dram_tensor("bv", [1, DG], BF16, kind="ExternalInput")
    woT = nc.dram_tensor("woT", [DG, DIM], BF16, kind="ExternalInput")
    ident = nc.dram_tensor("ident", [128, 128], BF16, kind="ExternalInput")
    outT = nc.dram_tensor("outT", [DIM, N], BF16, kind="ExternalOutput")

    with tile.TileContext(nc) as tc:
        _body(nc, tc, xT, x8, wq8, wk8, bqc, bkc, wvT, bv, woT, ident, outT)

    nc.compile()
    return nc


def _body(nc, tc, xT, x8, wq8, wk8, bqc, bkc, wvT, bv, woT, ident, outT):
    from contextlib import ExitStack

    Exp = mybir.ActivationFunctionType.Exp

    with ExitStack() as ctx:
        persist = ctx.enter_context(tc.tile_pool(name="persist", bufs=1))
        e_pool = ctx.enter_context(tc.tile_pool(name="e_sb", bufs=22))
        t_pool = ctx.enter_context(tc.tile_pool(name="t_sb", bufs=3))
        on_pool = ctx.enter_context(tc.tile_pool(name="on_sb", bufs=18))

        # --- input loads ----------------------------------------------------
        x8_sb, wq8_sb, wk8_sb = [], [], []
        for c in range(XC):
            t = persist.tile([128, 2 * N], FP8, tag=f"x8{c}", name=f"x8{c}")
            nc.sync.dma_start(out=t[:], in_=x8.ap()[c * 128:(c + 1) * 128, :])
            x8_sb.append(t.rearrange("p (j n) -> p j n", j=2))
            t = persist.tile([128, 512], FP8, tag=f"wq8{c}", name=f"wq8{c}")
            nc.sync.dma_start(out=t[:], in_=wq8.ap()[c * 128:(c + 1) * 128, :])
            wq8_sb.append(t.rearrange("p (j ji c) -> p j ji c", j=2, ji=2))
            t = persist.tile([128, 512], FP8, tag=f"wk8{c}", name=f"wk8{c}")
            nc.sync.dma_start(out=t[:], in_=wk8.ap()[c * 128:(c + 1) * 128, :])
            wk8_sb.append(t.rearrange("p (j ji c) -> p j ji c", j=2, ji=2))
        bq_sb = persist.tile([128, 2], F32, tag="bq", name="bq")
        nc.sync.dma_start(out=bq_sb[:], in_=bqc.ap()[:, :])
        bk_sb = persist.tile([128, 2], F32, tag="bk", name="bk")
        nc.sync.dma_start(out=bk_sb[:], in_=bkc.ap()[:, :])
        xt_sb = []
        for kc in range(8):
            t = persist.tile([128, N], BF16, tag=f"xt{kc}", name=f"xt{kc}")
            nc.sync.dma_start(out=t[:], in_=xT.ap()[kc * 128:(kc + 1) * 128, :])
            xt_sb.append(t)
        wv_sb = []
        for kc in range(8):
            t = persist.tile([128, DG], BF16, tag=f"wv{kc}", name=f"wv{kc}")
            nc.sync.dma_start(out=t[:], in_=wvT.ap()[kc * 128:(kc + 1) * 128, :])
            wv_sb.append(t)
        bv_sb = persist.tile([1, DG], BF16, tag="bv", name="bv")
        nc.sync.dma_start(out=bv_sb[:], in_=bv.ap()[:, :])
        id_sb = persist.tile([128, 128], BF16, tag="ident", name="ident")
        nc.sync.dma_start(out=id_sb[:], in_=ident.ap()[:, :])
        wo_sb = []
        for pc in range(2):
            t = persist.tile([128, DIM], BF16, tag=f"wo{pc}", name=f"wo{pc}")
            nc.sync.dma_start(out=t[:], in_=woT.ap()[pc * 128:(pc + 1) * 128, :])
            wo_sb.append(t)
        ones = persist.tile([1, 512], BF16, tag="ones", name="ones")
        nc.vector.memset(ones[:], 1.0)
        # warm the ScalarE Exp table while DMAs stream in
        warm = persist.tile([1, 1], F32, tag="warm", name="warm")
        nc.scalar.activation(warm[:], ones[:, 0:1], Exp)

        # persistent activations
        qt_t = [persist.tile([128, 2 * 512], FP8, tag=f"qt{nb}",
                             name=f"qt{nb}") for nb in range(NB)]
        kt_t = [persist.tile([128, 2 * 1024], FP8, tag=f"kt{i}",
                             name=f"kt{i}") for i in range(2)]
        qt_r = [t.rearrange("p (j n) -> p j n", j=2) for t in qt_t]
        kt_r = [t.rearrange("p (j n) -> p j n", j=2) for t in kt_t]
        v_sb = [persist.tile([128, HPG * 65], BF16, tag=f"v{mt}",
                             name=f"v{mt}") for mt in range(MT)]
        oT_sb = [persist.tile([128, N], BF16, tag=f"oT{p}", name=f"oT{p}")
                 for p in range(2)]
        recip = persist.tile([128, 16], F32, tag="recip", name="recip")

        # --- phase 1: q/k projections (fp8 DoubleRow) -----------------------
        # PSUM pass (tensor, jpass, nb) accumulates 4 DR chunk-matmuls; DVE
        # evicts with the per-partition bias straight into the fp8 pair
        # layout.  Order feeds kt for n-halves before qt token blocks.
        with tc.tile_pool(name="qk_ps", bufs=4, space="PSUM") as qkp:
            def proj_group(w_sb, bias_sb, dst, j, nb):
                ps = qkp.tile([128, 512], F32, tag="qkps", name="qkps")
                for c in range(XC):
                    nc.tensor.matmul(
                        ps[:],
                        lhsT=w_sb[c][:, j],
                        rhs=x8_sb[c][:, :, nb * 512:(nb + 1) * 512],
                        start=(c == 0), stop=(c == XC - 1), perf_mode=DR)
                nc.vector.tensor_scalar(
                    dst, ps[:], bias_sb[:, j:j + 1], None, Add)

            def kdst(j, nb):
                return kt_r[nb // 2][:, j, (nb % 2) * 512:(nb % 2 + 1) * 512]

            # k-half0 first (gates scores half0 of every head), then q nb0,
            # k-half1, then remaining q blocks.
            for j in range(2):
                proj_group(wk8_sb, bk_sb, kdst(j, 0), j, 0)
            for j in range(2):
                proj_group(wk8_sb, bk_sb, kdst(j, 1), j, 1)
            for j in range(2):
                proj_group(wq8_sb, bq_sb, qt_r[0][:, j, :], j, 0)
            for j in range(2):
                proj_group(wk8_sb, bk_sb, kdst(j, 2), j, 2)
            for j in range(2):
                proj_group(wk8_sb, bk_sb, kdst(j, 3), j, 3)
            for nb in (1, 2, 3):
                for j in range(2):
                    proj_group(wq8_sb, bq_sb, qt_r[nb][:, j, :], j, nb)

        # --- main pipeline ---------------------------------------------------
        s_pool_cm = tc.tile_pool(name="s_ps", bufs=2, space="PSUM")
        s_pool = s_pool_cm.__enter__()
        av_cm = tc.tile_pool(name="av_ps", bufs=1, space="PSUM")
        avp = av_cm.__enter__()
        avA = avp.tile([128, 455], F32, tag="avA", name="avA")
        avB = avp.tile([128, 455], F32, tag="avB", name="avB")
        avC = avp.tile([128, 130], F32, tag="avC", name="avC")
        avA_r = avA.rearrange("p (nb c) -> p nb c", c=65)
        avB_r = avB.rearrange("p (nb c) -> p nb c", c=65)
        avC_r = avC.rearrange("p (nb c) -> p nb c", c=65)

        def av_slice(nb):
            if nb < 7:
                return avA[:, nb * 65:(nb + 1) * 65]
            if nb < 14:
                return avB[:, (nb - 7) * 65:(nb - 6) * 65]
            return avC[:, (nb - 14) * 65:(nb - 13) * 65]

        e_tiles = {}

        def emit_s_exp(h, mt, half):
            s_ps = s_pool.tile([128, 1024], F32, tag="sps", name="sps")
            qsl = qt_r[mt // 4][32 * h:32 * h + 32, :,
                                 (mt % 4) * 128:(mt % 4 + 1) * 128]
            for jj in range(2):
                nc.tensor.matmul(
                    s_ps[:, jj * 512:(jj + 1) * 512],
                    lhsT=qsl,
                    rhs=kt_r[half][32 * h:32 * h + 32, :,
                                   jj * 512:(jj + 1) * 512],
                    start=True, stop=True, perf_mode=DR,
                    tile_position=(32 * h, 0))
            e = e_pool.tile([128, 1024], BF16, tag="e", name="e")
            if _unit_engine(h, mt, half) == "act":
                nc.scalar.activation(e[:], s_ps[:], Exp, scale=SCALE_EFF)
            else:
                t = t_pool.tile([128, 1024], BF16, tag="t", name="t")
                nc.vector.tensor_scalar(t[:], s_ps[:], TAYC, 1.0, Mult, Add)
                nc.gpsimd.tensor_mul(e[:], t[:], t[:])
            e_tiles[h, mt, half] = e

        def emit_av(h, mc):
            v_r = v_sb[mc].rearrange("p (h c) -> p h c", c=65)
            for nb in range(16):
                e = e_tiles[h, mc, nb // 8]
                # start=True zeroes the whole PSUM bank, so only the first
                # slice per bank (nb 0/7/14) carries it; siblings accumulate
                # onto the bank-zeroed state.
                nc.tensor.matmul(
                    av_slice(nb),
                    lhsT=e[:, (nb % 8) * 128:(nb % 8 + 1) * 128],
                    rhs=v_r[:, h, :],
                    start=(mc == 0 and nb in (0, 7, 14)),
                    stop=(mc == MT - 1))
            if mc == MT - 1:
                for half in range(2):
                    del e_tiles[h, mc, half]
            elif mc >= 1:
                for half in range(2):
                    del e_tiles[h, mc - 1, half]

        def emit_v(vps, mt):
            ps = vps.tile([128, DG], F32, tag="vps", name="vpsn")
            for kc in range(8):
                nc.tensor.matmul(
                    ps[:],
                    lhsT=xt_sb[kc][:, mt * 128:(mt + 1) * 128],
                    rhs=wv_sb[kc][:],
                    start=(kc == 0), stop=False)
            nc.tensor.matmul(
                ps[:], lhsT=ones[:, :128], rhs=bv_sb[:],
                start=False, stop=True)
            dst = v_sb[mt].rearrange("p (h c) -> p h c", c=65)
            nc.vector.tensor_copy(dst[:, :, 0:64],
                                  ps.rearrange("p (h c) -> p h c", c=64))
            nc.vector.memset(dst[:, :, 64:65], 1.0)

        def emit_norm(h):
            nc.vector.reciprocal(recip[:, 0:7], avA_r[:, :, 64])
            nc.vector.reciprocal(recip[:, 7:14], avB_r[:, :, 64])
            nc.vector.reciprocal(recip[:, 14:16], avC_r[:, :, 64])
            o_n = []
            for nb in range(16):
                t = on_pool.tile([128, 64], BF16, tag="on", name="on")
                nc.vector.tensor_scalar(
                    t[:], av_slice(nb)[:, 0:64], recip[:, nb:nb + 1], None,
                    Mult)
                o_n.append(t)
            return o_n

        def emit_tr_group(tr_pool, h, o_n, g):
            """transpose o_n[4g..4g+4) and evict to oT (one 512-col group)."""
            trt = tr_pool.tile([64, 512], BF16, tag="tr", name="tr")
            for i in range(4):
                nc.tensor.transpose(
                    trt[:, i * 128:(i + 1) * 128], o_n[4 * g + i][:], id_sb[:])
            p, hh = divmod(h, 2)
            nc.vector.tensor_copy(
                oT_sb[p][hh * 64:(hh + 1) * 64, g * 512:(g + 1) * 512], trt[:])

        # ---- head 0 (with v projection, av lagged) -------------------------
        vps_cm = tc.tile_pool(name="vps", bufs=1, space="PSUM")
        vps = vps_cm.__enter__()
        H0_VD, H0_LAG = 4, 8
        for mt in range(MT):
            emit_s_exp(0, mt, 0)
            if mt >= H0_VD:
                emit_v(vps, mt - H0_VD)
            emit_s_exp(0, mt, 1)
            if mt >= H0_LAG:
                emit_av(0, mt - H0_LAG)
        for j in range(MT - H0_VD, MT):
            emit_v(vps, j)
        for mc in range(MT - H0_LAG, MT):
            emit_av(0, mc)
        vps_cm.__exit__(None, None, None)
        tr_cm = tc.tile_pool(name="tr_ps", bufs=1, space="PSUM")
        tr_pool = tr_cm.__enter__()
        pend = (0, emit_norm(0))

        # ---- heads 1..3 ----------------------------------------------------
        for h in (1, 2, 3):
            for mt in range(MT):
                emit_s_exp(h, mt, 0)
                emit_s_exp(h, mt, 1)
                if mt >= AV_LAG:
                    emit_av(h, mt - AV_LAG)
                if pend is not None and AV_LAG <= mt < AV_LAG + 4:
                    ph, po = pend
                    emit_tr_group(tr_pool, ph, po, mt - AV_LAG)
                    if mt == AV_LAG + 3:
                        pend = None
            for mc in range(MT - AV_LAG, MT):
                emit_av(h, mc)
            pend = (h, emit_norm(h))

        # head-3 transposes before the PSUM pools unwind (LIFO release)
        ph, po = pend
        for g in range(4):
            emit_tr_group(tr_pool, ph, po, g)
        tr_cm.__exit__(None, None, None)
        av_cm.__exit__(None, None, None)
        s_pool_cm.__exit__(None, None, None)

        # ---- tail: output projection ---------------------------------------
        with (
            tc.tile_pool(name="out_ps", bufs=6, space="PSUM") as out_pool,
            tc.tile_pool(name="out_sb", bufs=6) as ostage,
        ):
            # ft-pairs share one [128, 1024] stage + one wide DMA (outT
            # row-pairs are contiguous), halving DMA latency overheads.
            for nb in range(NB):
                for fp in range(FT // 2):
                    stage = ostage.tile([128, 2 * 512], BF16, tag="ostage",
                                        name="ostage")
                    for fh in range(2):
                        ft = fp * 2 + fh
                        ps = out_pool.tile([128, 512], F32, tag="outps",
                                           name="outps")
                        for pc in range(2):
                            nc.tensor.matmul(
                                ps[:],
                                lhsT=wo_sb[pc][:, ft * 128:(ft + 1) * 128],
                                rhs=oT_sb[pc][:, nb * 512:(nb + 1) * 512],
                                start=(pc == 0), stop=(pc == 1))
                        if (nb * FT + ft) % 3 == 2:
                            nc.vector.tensor_copy(
                                stage[:, fh * 512:(fh + 1) * 512], ps[:])
                        else:
                            nc.scalar.copy(
                                stage[:, fh * 512:(fh + 1) * 512], ps[:])
                    nc.sync.dma_start(
                        out=outT.ap()[fp * 256:fp * 256 + 256,
                                      nb * 512:(nb + 1) * 512].rearrange(
                            "(f p) n -> p f n", f=2),
                        in_=stage.rearrange("p (f n) -> p f n", f=2))


_CACHED_NC = None


def _get_nc():
    global _CACHED_NC
    if _CACHED_NC is None:
        _CACHED_NC = build_kernel()
    return _CACHED_NC


def _pack_w8(WT_g, ws):
    """[1024, 256] slice of W.T (in-feat, out-feat) -> [512, 512] fp8 DR
    layout: row c*128+p_in, col jpass*256 + j_in*128 + (h*32 + r)."""
    A = (ws * WT_g).astype(np.float32)
    A3 = A.reshape(XC, 2, 128, HPG, 2, 32)      # c, j_in, p_in, h, jpass, r
    A3 = A3.transpose(0, 2, 4, 1, 3, 5)          # c, p_in, jpass, j_in, h, r
    return np.ascontiguousarray(A3.reshape(XC * 128, 512)).astype(NPFP8)


def _pack_b(b_g, ws):
    """[256] group bias -> [128, 2] f32: [p, jpass] = ws*b[64*(p//32) +
    jpass*32 + p%32]."""
    A = (ws * b_g).astype(np.float32).reshape(HPG, 2, 32)  # h, jpass, r
    return np.ascontiguousarray(A.transpose(0, 2, 1).reshape(128, 2))


def make_in_maps(x, Wq, bq, Wk, bk, Wv, bv, Wo, bo):
    """Host-side shard/layout prep: per-core input dict."""
    x = np.asarray(x, dtype=np.float32)
    xT_b = [np.ascontiguousarray(x[b].T) for b in range(B)]
    x8_b = []
    for b in range(B):
        xr = xT_b[b].reshape(XC, 2, 128, N)      # c, j, p, n
        xr = xr.transpose(0, 2, 1, 3)            # c, p, j, n
        x8_b.append(np.ascontiguousarray(xr.reshape(XC * 128, 2 * N))
                    .astype(NPFP8))
    WqT = np.asarray(Wq, np.float32).T  # [in-feat, out-feat]
    WkT = np.asarray(Wk, np.float32).T
    WvT = np.asarray(Wv, np.float32).T
    WoT = np.asarray(Wo, np.float32).T
    bq = np.asarray(bq, np.float32)
    bk = np.asarray(bk, np.float32)
    bv16 = np.asarray(bv, np.float32).astype(NPBF16)
    ident = np.eye(128, dtype=np.float32).astype(NPBF16)

    in_maps = []
    for c in range(N_CORES):
        b, g = divmod(c, GROUPS)
        sl = slice(g * DG, (g + 1) * DG)
        in_maps.append({
            "xT": xT_b[b].astype(NPBF16),
            "x8": x8_b[b],
            "wq8": _pack_w8(WqT[:, sl], WS),
            "wk8": _pack_w8(WkT[:, sl], WS),
            "bqc": _pack_b(bq[sl], WS),
            "bkc": _pack_b(bk[sl], WS),
            "wvT": np.ascontiguousarray(WvT[:, sl]).astype(NPBF16),
            "bv": bv16[sl].reshape(1, DG),
            "woT": np.ascontiguousarray(WoT[sl, :]).astype(NPBF16),
            "ident": ident,
        })
    return in_maps


def combine_outputs(results, bo):
    """Host-side unshard: sum group partials per batch, add bo."""
    bo = np.asarray(bo, np.float32)
    out = np.zeros((B, N, DIM), np.float32)
    for c in range(N_CORES):
        b = c // GROUPS
        out[b] += results[c]["outT"].astype(np.float32).T
    out += bo
    return out


def kernel(**inputs):
    nc = _get_nc()
    in_maps = make_in_maps(**{k: inputs[k] for k in
                              ("x", "Wq", "bq", "Wk", "bk", "Wv", "bv",
                               "Wo", "bo")})
    res = run_bass_kernel_spmd(nc, in_maps, list(range(N_CORES)))
    return combine_outputs(res.results, inputs["bo"])


if __name__ == "__main__":
    rng = np.random.default_rng(0)
    ins = {
        "x": rng.standard_normal((B, N, DIM), np.float32),
        "Wq": rng.standard_normal((DIM, DIM), np.float32) * 0.02,
        "bq": rng.standard_normal((DIM,), np.float32) * 0.02,
        "bk": rng.standard_normal((DIM,), np.float32) * 0.02,
        "Wk": rng.standard_normal((DIM, DIM), np.float32) * 0.02,
        "Wv": rng.standard_normal((DIM, DIM), np.float32) * 0.02,
        "bv": rng.standard_normal((DIM,), np.float32) * 0.02,
        "Wo": rng.standard_normal((DIM, DIM), np.float32) * 0.02,
        "bo": rng.standard_normal((DIM,), np.float32) * 0.02,
    }
    out = kernel(**ins)
    print("kernel output", out.shape, out.dtype, float(np.abs(out).mean()))
